# revision 29
# baseline (speedup 1.0000x reference)
"""Multi-head attention (SEQ=4096, EMBED=1024, 16 heads, Dh=64) on 8 TRN2
NeuronCores, head-parallel: 2 heads per core, Wo row-sharded so each core
emits a partial output [SEQ, EMBED] (fp16); the host sums the 8 partials
(+bo).

v3 design notes (vs the 402us baseline):
  - All 16-bit storage is fp16 (same PE cost as bf16, 8x finer mantissa).
  - The scalar-engine Exp over [128, 1024] score chunks is the hard floor
    (256 x ~1.1us back-to-back). Everything else hides under it:
      * AV matmuls lag the score/Exp stream by LAG chunks so the in-order
        PE queue never blocks on the sup-boundary drain.
      * softmax normalization happens once per sup: D (row 64 of the attn
        accumulator, from the ones column of V') is staged to fp16, PE
        K=1-broadcast to 64 partitions, reciprocal'd partition-parallel
        on DVE, and multiplied into xTn. The output projection then sums
        both heads inside one PSUM accumulation group.
  - hidden^T goes through the DMA xbar in [1024, 128] slabs alternating
    between the only two transpose-capable queues (SP, Activation); weight
    loads use contiguous host layouts on the vector/gpsimd queues so the
    transposes start immediately.
  - PSUM: sc 2x[128,1024] (4 banks) + at0/at1 [65,512] (2) + aux ring
    2x[128,512] (2) = 8 banks.
"""

import os
import sys

sys.path.insert(0, "/opt/trn_rl_repo")

import numpy as np

SEQ = 4096
EMBED = 1024
HEADS = 16
HD = 64
NCORES = 8
HPC = HEADS // NCORES  # 2 heads per core
EC = EMBED // 128  # 8 e-chunks
SUP = 512  # s-super size
NSUP = SEQ // SUP  # 8
TC = SEQ // 128  # 32 t-chunks
JS = SUP // 128  # 4 s-tiles per super
LAG = 8  # AV lag behind the score/Exp stream, in chunks

LAST = None  # BassKernelResults of the most recent run (read by test.py)
_CACHE = {}


def _build():
    import concourse.bacc as bacc
    import concourse.tile as tile
    from concourse import mybir

    f16 = mybir.dt.float16
    f32 = mybir.dt.float32

    nc = bacc.Bacc("TRN2", debug=False, enable_asserts=False, num_devices=NCORES)

    hid = nc.dram_tensor("hidden_f16", [SEQ, EMBED], f16, kind="ExternalInput").ap()
    wqkv = nc.dram_tensor("w_qkv", [128, 3, EC, 128], f16, kind="ExternalInput").ap()
    bqkv = nc.dram_tensor("b_qkv", [128, 3], f32, kind="ExternalInput").ap()
    wo = nc.dram_tensor("w_o", [64, HPC, EMBED], f16, kind="ExternalInput").ap()
    ident = nc.dram_tensor("ident", [128, 128], f16, kind="ExternalInput").ap()
    outp = nc.dram_tensor("out_p", [SEQ, EMBED], f16, kind="ExternalOutput").ap()

    with tile.TileContext(nc) as tc:
        _emit(tc, mybir, hid, wqkv, bqkv, wo, ident, outp)

    nc.compile()
    return nc


def _emit(tc, mybir, hid, wqkv, bqkv, wo, ident, outp):
    import contextlib

    import concourse.bass as bass

    nc = tc.nc
    ts = bass.ts
    f16 = mybir.dt.float16
    f32 = mybir.dt.float32
    Exp = mybir.ActivationFunctionType.Exp
    AOT = mybir.AluOpType

    st_ = contextlib.ExitStack()
    persist = st_.enter_context(tc.tile_pool(name="persist", bufs=1))
    hTa = persist.tile([128, EC, SEQ], f16, tag="hTa")  # hidden^T, all chunks
    qT = persist.tile([128, SEQ], f16, tag="qT")  # [(h,d), s]
    kT = persist.tile([128, SEQ], f16, tag="kT")
    vP = persist.tile([128, TC, HPC * (HD + 1)], f16, tag="vP")  # V' per t-chunk
    # normalized attn^T per head: [d, s] on partitions 0-63
    xTn = [
        persist.tile([HD, SEQ], f16, tag=f"xTn{h}", name=f"xTn{h}")
        for h in range(HPC)
    ]
    wqkv_sb = persist.tile([128, 3, EC, 128], f16, tag="wqkv")
    wq_sb, wk_sb, wv_sb = (wqkv_sb[:, i] for i in range(3))
    wo_sb = persist.tile([64, HPC, EMBED], f16, tag="wo")  # head-major Wo rows
    id_sb = persist.tile([128, 128], f16, tag="ident")
    bqkv_sb = persist.tile([128, 3], f32, tag="bqkv")
    bq_sb, bk_sb, bv_sb = (bqkv_sb[:, i : i + 1] for i in range(3))
    ones_sb = persist.tile([1, HD], f16, tag="ones")

    # hidden^T via DMA xbar in [1024, 128] slabs (pair p = s-blocks 2p, 2p+1),
    # alternating between the two transpose-capable queues.
    def ht_pair(p):
        for c in range(EC):
            eng = nc.sync
            eng.dma_start(
                out=hTa[:, c, ts(p, 2 * SUP)],
                in_=hid[ts(p, 2 * SUP), ts(c, 128)],
                transpose=True,
            )

    def ht_blk(b):
        for c in range(EC):
            nc.sync.dma_start(
                out=hTa[:, c, ts(b, SUP)],
                in_=hid[ts(b, SUP), ts(c, 128)],
                transpose=True,
            )

    # one dense sync-queue stream ordered by first use: q/k weights, the
    # block-0 slabs (with wv/biases woven in), then blocks 1..7
    def ht_one(b, c):
        nc.sync.dma_start(
            out=hTa[:, c, ts(b, SUP)],
            in_=hid[ts(b, SUP), ts(c, 128)],
            transpose=True,
        )

    nc.sync.dma_start(out=wqkv_sb, in_=wqkv)
    nc.sync.dma_start(out=bqkv_sb, in_=bqkv)
    nc.sync.dma_start(out=id_sb, in_=ident)
    nc.sync.dma_start(out=wo_sb, in_=wo)
    ht_pair(0)
    ht_pair(1)
    ht_pair(2)
    ht_pair(3)
    nc.vector.memset(ones_sb, 1.0)
    # ones columns of V' (free position h*65+64); V fills cols 0..63
    vP_ones = vP.rearrange("p c (h e) -> p c h e", h=HPC)[:, :, :, HD : HD + 1]
    ones_col = persist.tile([128, 1], f16, tag="ones_col")
    nc.vector.memset(ones_col, 1.0)
    ones_b = bass.AP(
        tensor=ones_col.tensor,
        offset=ones_col.offset,
        ap=[ones_col.ap[0], [0, TC], [0, HPC], [0, 1]],
    )
    nc.vector.tensor_copy(out=vP_ones, in_=ones_b)

    # ---- pools -------------------------------------------------------
    pT_p = st_.enter_context(tc.tile_pool(name="pT", bufs=LAG + 5))
    vT_p = st_.enter_context(tc.tile_pool(name="vT", bufs=3))
    dn_p = st_.enter_context(tc.tile_pool(name="dn", bufs=3))
    ostg_p = st_.enter_context(tc.tile_pool(name="ostg", bufs=4))
    sc_ps_p = st_.enter_context(tc.tile_pool(name="ps_sc", bufs=2, space="PSUM"))
    at_ps_p = st_.enter_context(tc.tile_pool(name="ps_at", bufs=1, space="PSUM"))
    aux_ps_p = st_.enter_context(tc.tile_pool(name="ps_aux", bufs=2, space="PSUM"))

    at_of = {}
    pT_of = {}
    kv_ps = {}

    # ---- phase A primitives ------------------------------------------
    def q_proj(sup, half=None):
        if half in (None, 0):
            q_ps = aux_ps_p.tile([128, SUP], f32, tag="aux", name=f"q_ps{sup}")
            kv_ps[("q", sup)] = q_ps
        else:
            q_ps = kv_ps.pop(("q", sup))
        rng = range(EC) if half is None else (
            range(EC // 2) if half == 0 else range(EC // 2, EC)
        )
        for c in rng:
            nc.tensor.matmul(
                q_ps, wq_sb[:, c, :], hTa[:, c, ts(sup, SUP)],
                start=(c == 0), stop=(c == EC - 1),
            )
        if half in (None, 1):
            nc.vector.tensor_scalar(
                out=qT[:, ts(sup, SUP)], in0=q_ps,
                scalar1=bq_sb, scalar2=None, op0=AOT.add,
            )

    def k_part(b):
        k_ps = aux_ps_p.tile([128, SUP], f32, tag="aux", name=f"k_ps{b}")
        for c in range(EC):
            nc.tensor.matmul(
                k_ps, wk_sb[:, c, :], hTa[:, c, ts(b, SUP)],
                start=(c == 0), stop=(c == EC - 1),
            )
        nc.vector.tensor_scalar(
            out=kT[:, ts(b, SUP)], in0=k_ps,
            scalar1=bk_sb, scalar2=None, op0=AOT.add,
        )

    def v_part(b):
        vT_ps = aux_ps_p.tile([128, SUP], f32, tag="aux", name=f"vT_ps{b}")
        for c in range(EC):
            nc.tensor.matmul(
                vT_ps, wv_sb[:, c, :], hTa[:, c, ts(b, SUP)],
                start=(c == 0), stop=(c == EC - 1),
            )
        vT_sb = vT_p.tile([128, SUP], f16, tag="vT", name=f"vT{b}")
        nc.vector.tensor_scalar(
            out=vT_sb, in0=vT_ps, scalar1=bv_sb, scalar2=None, op0=AOT.add
        )
        tp_ps = aux_ps_p.tile([128, JS, 128], f16, tag="aux", name=f"tp_ps{b}")
        for j in range(JS):
            nc.tensor.transpose(tp_ps[:, j, :], vT_sb[:, ts(j, 128)], id_sb)
        for j in range(JS):
            t_idx = JS * b + j
            dst = vP[:, t_idx, :].rearrange("p (h e) -> p h e", h=HPC)[:, :, 0:HD]
            nc.vector.tensor_copy(
                out=dst,
                in_=tp_ps[:, j, :].rearrange("p (h d) -> p h d", h=HPC),
            )

    # ---- phase B primitives ------------------------------------------
    def sc_exp(sup, c):
        sc_ps = sc_ps_p.tile([128, HPC * SUP], f32, tag="sc", name=f"sc{sup}_{c}")
        for h in range(HPC):
            nc.tensor.matmul(
                sc_ps[:, ts(h, SUP)],
                kT[ts(h, HD), ts(c, 128)],
                qT[ts(h, HD), ts(sup, SUP)],
                start=True, stop=True,
                tile_position=(h * HD, 0),
            )
        pT = pT_p.tile([128, HPC * SUP], f16, tag="pT", name=f"pT{sup}_{c}")
        nc.scalar.activation(out=pT, in_=sc_ps, func=Exp)
        pT_of[(sup, c)] = pT

    def at_mms(sup, c):
        pT = pT_of.pop((sup, c))
        for h in range(HPC):
            nc.tensor.matmul(
                at_of[sup][h],
                vP[:, c, ts(h, HD + 1)],
                pT[:, ts(h, SUP)],
                start=(c == 0), stop=(c == TC - 1),
            )

    def alloc_at(sup):
        at_of[sup] = [
            at_ps_p.tile([HD + 1, SUP], f32, tag=f"at{h}", name=f"at{sup}_{h}")
            for h in range(HPC)
        ]

    # ---- phase C: normalize + output projection ----------------------
    def drain(sup, tail=False):
        # stage D (fp16) and raw attn out of PSUM, then normalize:
        # rb = D broadcast to 64 partitions via K=1 PE matmul, fast recip
        at = at_of[sup]
        cp = nc.scalar.copy if tail else nc.vector.tensor_copy
        d16 = dn_p.tile([1, HPC, SUP], f16, tag="d16", name=f"d16_{sup}")
        for h in range(HPC):
            cp(out=d16[:, h, :], in_=at[h][HD : HD + 1, :])
        xstg = dn_p.tile([HD, HPC, SUP], f32, tag="xstg", name=f"xstg{sup}")
        for h in range(HPC):
            cp(out=xstg[:, h, :], in_=at[h][0:HD, :])
        rb_ps = [
            aux_ps_p.tile([HD, SUP], f32, tag="aux", name=f"rb{sup}_{h}")
            for h in range(HPC)
        ]
        for h in range(HPC):
            nc.tensor.matmul(
                rb_ps[h], ones_sb, d16[:, h, :], start=True, stop=True
            )
        rb = dn_p.tile([HD, HPC, SUP], f32, tag="rb", name=f"rb{sup}")
        for h in range(HPC):
            nc.vector.reciprocal_approx_fast(out=rb[:, h, :], in_=rb_ps[h])
        for h in range(HPC):
            nc.vector.tensor_mul(
                out=xTn[h][:, ts(sup, SUP)], in0=xstg[:, h, :], in1=rb[:, h, :]
            )

    def out_unit(sup, j):
        # one 128-row output chunk: o = sum_h xTn[h].T @ Wo_h (PSUM-fused)
        st_i = JS * sup + j
        o_ps = [
            aux_ps_p.tile([128, SUP], f32, tag="aux", name=f"o{st_i}_{eh}")
            for eh in range(EMBED // SUP)
        ]
        for eh in range(EMBED // SUP):
            for h in range(HPC):
                nc.tensor.matmul(
                    o_ps[eh],
                    xTn[h][:, ts(st_i, 128)],
                    wo_sb[:, h, ts(eh, SUP)],
                    start=(h == 0), stop=(h == HPC - 1),
                )
        stage = ostg_p.tile([128, EMBED], f16, tag="ostg", name=f"ostg{st_i}")
        for eh in range(EMBED // SUP):
            nc.vector.tensor_copy(out=stage[:, ts(eh, SUP)], in_=o_ps[eh])
        nc.gpsimd.dma_start(out=outp[ts(st_i, 128), :], in_=stage)

    # ---- global conveyor ---------------------------------------------
    # score/Exp stream at (sup, c); the AV stream lags by LAG=8 chunks.
    # At a sup boundary the previous sup's trailing 8 AVs run 2-per-chunk
    # over c=0..3, the drain at c==4, and the new sup's AVs start at c==8
    # -- so the PSUM handoff of the at accumulators never stalls the PE.
    # block-0 projections chase the pair-0 slabs together, e-chunk by
    # e-chunk (q rides the idle at-pool bank; aux ring holds k and v)
    k0_ps = aux_ps_p.tile([128, SUP], f32, tag="aux", name="k0_ps")
    v0_ps = aux_ps_p.tile([128, SUP], f32, tag="aux", name="v0_ps")
    q0_ps = at_ps_p.tile([128, SUP], f32, tag="at0", name="q0_ps")
    for e in range(EC):
        for w_sb, ps in ((wk_sb, k0_ps), (wv_sb, v0_ps), (wq_sb, q0_ps)):
            nc.tensor.matmul(
                ps, w_sb[:, e, :], hTa[:, e, ts(0, SUP)],
                start=(e == 0), stop=(e == EC - 1),
            )
    nc.vector.tensor_scalar(
        out=kT[:, ts(0, SUP)], in0=k0_ps,
        scalar1=bk_sb, scalar2=None, op0=AOT.add,
    )
    vT0_sb = vT_p.tile([128, SUP], f16, tag="vT", name="vT0")
    nc.vector.tensor_scalar(
        out=vT0_sb, in0=v0_ps, scalar1=bv_sb, scalar2=None, op0=AOT.add
    )
    nc.vector.tensor_scalar(
        out=qT[:, ts(0, SUP)], in0=q0_ps,
        scalar1=bq_sb, scalar2=None, op0=AOT.add,
    )
    tp0_ps = aux_ps_p.tile([128, JS, 128], f16, tag="aux", name="tp0_ps")
    for j in range(JS):
        nc.tensor.transpose(tp0_ps[:, j, :], vT0_sb[:, ts(j, 128)], id_sb)
    for j in range(JS):
        dst = vP[:, j, :].rearrange("p (h e) -> p h e", h=HPC)[:, :, 0:HD]
        nc.vector.tensor_copy(
            out=dst, in_=tp0_ps[:, j, :].rearrange("p (h d) -> p h d", h=HPC)
        )

    UNITS = (10, 15, 20, 25)
    for g in range(NSUP * TC):
        sup, c = divmod(g, TC)
        sc_exp(sup, c)
        if sup == 0 and c < 28:
            # phase A, fine-grained: block b = c//4+1 spreads its k/v
            # matmuls 2-at-a-time over the 4 chunks of block b-1, so a
            # late hTa slab never blocks more than one conveyor chunk.
            b = c // JS + 1
            i = c % JS
            if i == 0:
                kv_ps[b] = (
                    aux_ps_p.tile([128, SUP], f32, tag="aux", name=f"k_ps{b}"),
                    aux_ps_p.tile([128, SUP], f32, tag="aux", name=f"v_ps{b}"),
                )
            k_ps, v_ps = kv_ps[b]
            for e in (2 * i, 2 * i + 1):
                nc.tensor.matmul(
                    k_ps, wk_sb[:, e, :], hTa[:, e, ts(b, SUP)],
                    start=(e == 0), stop=(e == EC - 1),
                )
            for e in (2 * i, 2 * i + 1):
                nc.tensor.matmul(
                    v_ps, wv_sb[:, e, :], hTa[:, e, ts(b, SUP)],
                    start=(e == 0), stop=(e == EC - 1),
                )
            if i == JS - 1:
                nc.vector.tensor_scalar(
                    out=kT[:, ts(b, SUP)], in0=k_ps,
                    scalar1=bk_sb, scalar2=None, op0=AOT.add,
                )
                vT_sb = vT_p.tile([128, SUP], f16, tag="vT", name=f"vT{b}")
                nc.vector.tensor_scalar(
                    out=vT_sb, in0=v_ps, scalar1=bv_sb, scalar2=None, op0=AOT.add
                )
                tp_ps = aux_ps_p.tile(
                    [128, JS, 128], f16, tag="aux", name=f"tp_ps{b}"
                )
                for j in range(JS):
                    nc.tensor.transpose(tp_ps[:, j, :], vT_sb[:, ts(j, 128)], id_sb)
                for j in range(JS):
                    t_idx = JS * b + j
                    dst = vP[:, t_idx, :].rearrange(
                        "p (h e) -> p h e", h=HPC
                    )[:, :, 0:HD]
                    nc.vector.tensor_copy(
                        out=dst,
                        in_=tp_ps[:, j, :].rearrange("p (h d) -> p h d", h=HPC),
                    )
                del kv_ps[b]
        if sup >= 1 and c in UNITS:
            out_unit(sup - 1, UNITS.index(c))
        if c == 27 and sup + 1 < NSUP:
            q_proj(sup + 1, half=0)
        if c == 29 and sup + 1 < NSUP:
            q_proj(sup + 1, half=1)
        if sup >= 1:
            if c < JS:
                at_mms(sup - 1, TC - 2 * JS + 2 * c)
                at_mms(sup - 1, TC - 2 * JS + 2 * c + 1)
            elif c == JS:
                drain(sup - 1)
        if c == LAG:
            alloc_at(sup)
        if c >= LAG:
            at_mms(sup, c - LAG)
    # tail: trailing AVs with the final drain interleaved per head so the
    # normalize chain starts as soon as each head's accumulator closes
    S7 = NSUP - 1
    for k in range(JS - 1):
        at_mms(S7, TC - 2 * JS + 2 * k)
        at_mms(S7, TC - 2 * JS + 2 * k + 1)
    at_mms(S7, TC - 2)
    at7 = at_of[S7]
    d16t = dn_p.tile([1, HPC, SUP], f16, tag="d16", name="d16_t")
    xstgt = dn_p.tile([HD, HPC, SUP], f32, tag="xstg", name="xstg_t")
    rbt = dn_p.tile([HD, HPC, SUP], f32, tag="rb", name="rb_t")
    pT31 = pT_of.pop((S7, TC - 1))
    rb_pst = []
    for h in range(HPC):
        nc.tensor.matmul(
            at7[h], vP[:, TC - 1, ts(h, HD + 1)], pT31[:, ts(h, SUP)],
            start=False, stop=True,
        )
        nc.scalar.copy(out=d16t[:, h, :], in_=at7[h][HD : HD + 1, :])
        nc.scalar.copy(out=xstgt[:, h, :], in_=at7[h][0:HD, :])
        ps = aux_ps_p.tile([HD, SUP], f32, tag="aux", name=f"rbt{h}")
        nc.tensor.matmul(ps, ones_sb, d16t[:, h, :], start=True, stop=True)
        rb_pst.append(ps)
    for h in range(HPC):
        nc.vector.reciprocal_approx_fast(out=rbt[:, h, :], in_=rb_pst[h])
        nc.vector.tensor_mul(
            out=xTn[h][:, ts(S7, SUP)], in0=xstgt[:, h, :], in1=rbt[:, h, :]
        )
    for j in range(JS):
        st_i = JS * S7 + j
        sct = sc_ps_p.tile([128, HPC * SUP], f32, tag="sc", name=f"osc{j}")
        stage = ostg_p.tile([128, EMBED], f16, tag="ostg", name=f"ostgt{j}")
        for eh in range(EMBED // SUP):
            for h in range(HPC):
                nc.tensor.matmul(
                    sct[:, ts(eh, SUP)],
                    xTn[h][:, ts(st_i, 128)],
                    wo_sb[:, h, ts(eh, SUP)],
                    start=(h == 0), stop=(h == HPC - 1),
                )
            # drain each 512-wide bank as soon as its head-sum closes;
            # scalar + DVE split, halves DMA'd densely on the idle sync queue
            cp = nc.scalar.copy if eh == 0 else nc.vector.tensor_copy
            cp(out=stage[:, ts(eh, SUP)], in_=sct[:, ts(eh, SUP)])
            nc.sync.dma_start(
                out=outp[ts(st_i, 128), ts(eh, SUP)], in_=stage[:, ts(eh, SUP)]
            )

    st_.close()


def _shards(inputs):
    """Host-side prep: per-core input dicts (head-parallel, Wo row-shard)."""
    hs = np.asarray(inputs["hidden_state"], np.float32)
    Wq = np.asarray(inputs["Wq"], np.float32) * 0.125  # fold 1/sqrt(64); exact
    bq = np.asarray(inputs["bq"], np.float32) * 0.125
    Wk = np.asarray(inputs["Wk"], np.float32)
    bk = np.asarray(inputs["bk"], np.float32)
    Wv = np.asarray(inputs["Wv"], np.float32)
    bv = np.asarray(inputs["bv"], np.float32)
    Wo = np.asarray(inputs["Wo"], np.float32)
    ident = np.eye(128, dtype=np.float16)
    hs16 = np.ascontiguousarray(hs.astype(np.float16))

    in_maps = []
    for c in range(NCORES):
        h0 = HPC * c

        # [H,E,Dh] head-pair -> [E, 2*Dh] -> [128(e), EC, 128] partition-major
        def _w(W):
            w = np.transpose(W[h0 : h0 + HPC], (1, 0, 2)).reshape(EMBED, 128)
            w = w.reshape(EC, 128, 128).transpose(1, 0, 2)  # [e, chunk, d]
            return np.ascontiguousarray(w.astype(np.float16))

        w_qkv = np.stack([_w(Wq), _w(Wk), _w(Wv)], axis=1)
        b_qkv = np.stack(
            [
                bq[h0 : h0 + HPC].reshape(128),
                bk[h0 : h0 + HPC].reshape(128),
                bv[h0 : h0 + HPC].reshape(128),
            ],
            axis=1,
        )
        # Wo rows for this core's heads: [128, E] -> [64, 2, E] head-major
        w_o = np.ascontiguousarray(
            Wo[128 * c : 128 * (c + 1)]
            .reshape(HPC, 64, EMBED)
            .transpose(1, 0, 2)
            .astype(np.float16)
        )
        in_maps.append(
            {
                "hidden_f16": hs16,
                "w_qkv": np.ascontiguousarray(w_qkv),
                "b_qkv": np.ascontiguousarray(b_qkv),
                "w_o": w_o,
                "ident": ident,
            }
        )
    return in_maps


def kernel(**inputs):
    global LAST
    from concourse import bass_utils

    trace = bool(int(os.environ.get("K_TRACE", "0")))
    if trace:
        _install_ntff_shim()

    if "nc" not in _CACHE:
        _CACHE["nc"] = _build()
    nc = _CACHE["nc"]

    in_maps = _shards(inputs)
    res = bass_utils.run_bass_kernel_spmd(
        nc, in_maps, core_ids=list(range(NCORES)), trace=trace
    )
    LAST = res

    out = np.zeros((SEQ, EMBED), np.float64)
    for c in range(NCORES):
        out += res.results[c]["out_p"].astype(np.float64)
    out += np.asarray(inputs["bo"], np.float32).astype(np.float64)
    return out.astype(np.float32)


def _install_ntff_shim():
    """antenv.axon_hooks is absent from this image; recreate it so
    run_bass_kernel_spmd(trace=True) can reach the NTFF profiling hook."""
    import types

    if "antenv.axon_hooks" in sys.modules:
        return
    try:
        if "/root/.axon_site" not in sys.path:
            sys.path.insert(0, "/root/.axon_site")
        from trn_agent_boot.trn_boot import _ntff_profile_via_ctypes

        hook = _ntff_profile_via_ctypes("/opt/axon/libaxon_pjrt.so")
    except Exception:
        hook = None
    mod = types.ModuleType("antenv.axon_hooks")
    mod._hook = hook
    mod.get_axon_ntff_profile_hook = lambda: mod._hook
    mod.set_axon_ntff_profile_hook = lambda h: setattr(mod, "_hook", h)
    sys.modules["antenv.axon_hooks"] = mod


# revision 30
# speedup vs baseline: 1.1848x; 1.1848x over previous
"""Multi-head attention (SEQ=4096, EMBED=1024, 16 heads, Dh=64) on 8 TRN2
NeuronCores, head-parallel: 2 heads per core, Wo row-sharded so each core
emits a partial output [SEQ, EMBED] (fp16); the host sums the 8 partials
(+bo).

v3 design notes (vs the 402us baseline):
  - All 16-bit storage is fp16 (same PE cost as bf16, 8x finer mantissa).
  - The scalar-engine Exp over [128, 1024] score chunks is the hard floor
    (256 x ~1.1us back-to-back). Everything else hides under it:
      * AV matmuls lag the score/Exp stream by LAG chunks so the in-order
        PE queue never blocks on the sup-boundary drain.
      * softmax normalization happens once per sup: D (row 64 of the attn
        accumulator, from the ones column of V') is staged to fp16, PE
        K=1-broadcast to 64 partitions, reciprocal'd partition-parallel
        on DVE, and multiplied into xTn. The output projection then sums
        both heads inside one PSUM accumulation group.
  - hidden^T goes through the DMA xbar in [1024, 128] slabs alternating
    between the only two transpose-capable queues (SP, Activation); weight
    loads use contiguous host layouts on the vector/gpsimd queues so the
    transposes start immediately.
  - PSUM: sc 2x[128,1024] (4 banks) + at0/at1 [65,512] (2) + aux ring
    2x[128,512] (2) = 8 banks.
"""

import os
import sys

sys.path.insert(0, "/opt/trn_rl_repo")

import numpy as np

SEQ = 4096
EMBED = 1024
HEADS = 16
HD = 64
NCORES = 8
HPC = HEADS // NCORES  # 2 heads per core
EC = EMBED // 128  # 8 e-chunks
SUP = 512  # s-super size
NSUP = SEQ // SUP  # 8
TC = SEQ // 128  # 32 t-chunks
JS = SUP // 128  # 4 s-tiles per super
LAG = 8  # AV lag behind the score/Exp stream, in chunks

LAST = None  # BassKernelResults of the most recent run (read by test.py)
_CACHE = {}


def _build():
    import concourse.bacc as bacc
    import concourse.tile as tile
    from concourse import mybir

    f16 = mybir.dt.float16
    f32 = mybir.dt.float32

    nc = bacc.Bacc("TRN2", debug=False, enable_asserts=False, num_devices=NCORES)

    hid = nc.dram_tensor("hidden_f16", [SEQ, EMBED], f16, kind="ExternalInput").ap()
    wqkv = nc.dram_tensor("w_qkv", [128, 3, EC, 128], f16, kind="ExternalInput").ap()
    bqkv = nc.dram_tensor("b_qkv", [128, 3], f32, kind="ExternalInput").ap()
    wo = nc.dram_tensor("w_o", [64, HPC, EMBED], f16, kind="ExternalInput").ap()
    ident = nc.dram_tensor("ident", [128, 128], f16, kind="ExternalInput").ap()
    outp = nc.dram_tensor("out_p", [SEQ, EMBED], f16, kind="ExternalOutput").ap()

    with tile.TileContext(nc) as tc:
        _emit(tc, mybir, hid, wqkv, bqkv, wo, ident, outp)

    nc.compile()
    return nc


def _emit(tc, mybir, hid, wqkv, bqkv, wo, ident, outp):
    import contextlib

    import concourse.bass as bass

    nc = tc.nc
    ts = bass.ts
    f16 = mybir.dt.float16
    f32 = mybir.dt.float32
    Exp = mybir.ActivationFunctionType.Exp
    AOT = mybir.AluOpType

    st_ = contextlib.ExitStack()
    persist = st_.enter_context(tc.tile_pool(name="persist", bufs=1))
    hTa = persist.tile([128, EC, SEQ], f16, tag="hTa")  # hidden^T, all chunks
    qT = persist.tile([128, SEQ], f16, tag="qT")  # [(h,d), s]
    kT = persist.tile([128, SEQ], f16, tag="kT")
    vP = persist.tile([128, TC, HPC * (HD + 1)], f16, tag="vP")  # V' per t-chunk
    # normalized attn^T per head: [d, s] on partitions 0-63
    xTn = [
        persist.tile([HD, SEQ], f16, tag=f"xTn{h}", name=f"xTn{h}")
        for h in range(HPC)
    ]
    wqkv_sb = persist.tile([128, 3, EC, 128], f16, tag="wqkv")
    wq_sb, wk_sb, wv_sb = (wqkv_sb[:, i] for i in range(3))
    wo_sb = persist.tile([64, HPC, EMBED], f16, tag="wo")  # head-major Wo rows
    id_sb = persist.tile([128, 128], f16, tag="ident")
    bqkv_sb = persist.tile([128, 3], f32, tag="bqkv")
    bq_sb, bk_sb, bv_sb = (bqkv_sb[:, i : i + 1] for i in range(3))
    ones_sb = persist.tile([1, HD], f16, tag="ones")

    # hidden^T via DMA xbar in [1024, 128] slabs (pair p = s-blocks 2p, 2p+1),
    # alternating between the two transpose-capable queues.
    def ht_pair(p):
        for c in range(EC):
            eng = nc.sync
            eng.dma_start(
                out=hTa[:, c, ts(p, 2 * SUP)],
                in_=hid[ts(p, 2 * SUP), ts(c, 128)],
                transpose=True,
            )

    def ht_blk(b):
        for c in range(EC):
            nc.sync.dma_start(
                out=hTa[:, c, ts(b, SUP)],
                in_=hid[ts(b, SUP), ts(c, 128)],
                transpose=True,
            )

    # one dense sync-queue stream ordered by first use: q/k weights, the
    # block-0 slabs (with wv/biases woven in), then blocks 1..7
    def ht_one(b, c):
        nc.sync.dma_start(
            out=hTa[:, c, ts(b, SUP)],
            in_=hid[ts(b, SUP), ts(c, 128)],
            transpose=True,
        )

    nc.sync.dma_start(out=wqkv_sb, in_=wqkv)
    nc.sync.dma_start(out=bqkv_sb, in_=bqkv)
    nc.sync.dma_start(out=id_sb, in_=ident)
    nc.sync.dma_start(out=wo_sb, in_=wo)
    ht_pair(0)
    ht_pair(1)
    ht_pair(2)
    ht_pair(3)
    nc.vector.memset(ones_sb, 1.0)
    # ones columns of V' (free position h*65+64); V fills cols 0..63
    vP_ones = vP.rearrange("p c (h e) -> p c h e", h=HPC)[:, :, :, HD : HD + 1]
    ones_col = persist.tile([128, 1], f16, tag="ones_col")
    nc.vector.memset(ones_col, 1.0)
    ones_b = bass.AP(
        tensor=ones_col.tensor,
        offset=ones_col.offset,
        ap=[ones_col.ap[0], [0, TC], [0, HPC], [0, 1]],
    )
    nc.vector.tensor_copy(out=vP_ones, in_=ones_b)

    # ---- pools -------------------------------------------------------
    pT_p = st_.enter_context(tc.tile_pool(name="pT", bufs=LAG + 3))
    vT_p = st_.enter_context(tc.tile_pool(name="vT", bufs=2))
    dn_p = st_.enter_context(tc.tile_pool(name="dn", bufs=2))
    ostg_p = st_.enter_context(tc.tile_pool(name="ostg", bufs=3))
    sc_ps_p = st_.enter_context(tc.tile_pool(name="ps_sc", bufs=2, space="PSUM"))
    at_ps_p = st_.enter_context(tc.tile_pool(name="ps_at", bufs=1, space="PSUM"))
    aux_ps_p = st_.enter_context(tc.tile_pool(name="ps_aux", bufs=2, space="PSUM"))

    at_of = {}
    pT_of = {}
    kv_ps = {}

    # ---- phase A primitives ------------------------------------------
    def q_proj(sup, half=None):
        if half in (None, 0):
            q_ps = aux_ps_p.tile([128, SUP], f32, tag="aux", name=f"q_ps{sup}")
            kv_ps[("q", sup)] = q_ps
        else:
            q_ps = kv_ps.pop(("q", sup))
        rng = range(EC) if half is None else (
            range(EC // 2) if half == 0 else range(EC // 2, EC)
        )
        for c in rng:
            nc.tensor.matmul(
                q_ps, wq_sb[:, c, :], hTa[:, c, ts(sup, SUP)],
                start=(c == 0), stop=(c == EC - 1),
            )
        if half in (None, 1):
            nc.vector.tensor_scalar(
                out=qT[:, ts(sup, SUP)], in0=q_ps,
                scalar1=bq_sb, scalar2=None, op0=AOT.add,
            )

    def k_part(b):
        k_ps = aux_ps_p.tile([128, SUP], f32, tag="aux", name=f"k_ps{b}")
        for c in range(EC):
            nc.tensor.matmul(
                k_ps, wk_sb[:, c, :], hTa[:, c, ts(b, SUP)],
                start=(c == 0), stop=(c == EC - 1),
            )
        nc.vector.tensor_scalar(
            out=kT[:, ts(b, SUP)], in0=k_ps,
            scalar1=bk_sb, scalar2=None, op0=AOT.add,
        )

    def v_part(b):
        vT_ps = aux_ps_p.tile([128, SUP], f32, tag="aux", name=f"vT_ps{b}")
        for c in range(EC):
            nc.tensor.matmul(
                vT_ps, wv_sb[:, c, :], hTa[:, c, ts(b, SUP)],
                start=(c == 0), stop=(c == EC - 1),
            )
        vT_sb = vT_p.tile([128, SUP], f16, tag="vT", name=f"vT{b}")
        nc.vector.tensor_scalar(
            out=vT_sb, in0=vT_ps, scalar1=bv_sb, scalar2=None, op0=AOT.add
        )
        tp_ps = aux_ps_p.tile([128, JS, 128], f16, tag="aux", name=f"tp_ps{b}")
        for j in range(JS):
            nc.tensor.transpose(tp_ps[:, j, :], vT_sb[:, ts(j, 128)], id_sb)
        for j in range(JS):
            t_idx = JS * b + j
            dst = vP[:, t_idx, :].rearrange("p (h e) -> p h e", h=HPC)[:, :, 0:HD]
            nc.vector.tensor_copy(
                out=dst,
                in_=tp_ps[:, j, :].rearrange("p (h d) -> p h d", h=HPC),
            )

    # ---- phase B primitives ------------------------------------------
    def sc_exp(sup, c):
        sc_ps = sc_ps_p.tile([128, HPC * SUP], f32, tag="sc", name=f"sc{sup}_{c}")
        for h in range(HPC):
            nc.tensor.matmul(
                sc_ps[:, ts(h, SUP)],
                kT[ts(h, HD), ts(c, 128)],
                qT[ts(h, HD), ts(sup, SUP)],
                start=True, stop=True,
                tile_position=(h * HD, 0),
            )
        pT = pT_p.tile([128, HPC * SUP], f16, tag="pT", name=f"pT{sup}_{c}")
        nc.scalar.activation(out=pT, in_=sc_ps, func=Exp)
        pT_of[(sup, c)] = pT

    def at_mms(sup, c):
        pT = pT_of.pop((sup, c))
        for h in range(HPC):
            nc.tensor.matmul(
                at_of[sup][h],
                vP[:, c, ts(h, HD + 1)],
                pT[:, ts(h, SUP)],
                start=(c == 0), stop=(c == TC - 1),
            )

    def alloc_at(sup):
        at_of[sup] = [
            at_ps_p.tile([HD + 1, SUP], f32, tag=f"at{h}", name=f"at{sup}_{h}")
            for h in range(HPC)
        ]

    # ---- phase C: normalize + output projection ----------------------
    def drain(sup, tail=False):
        # stage D (fp16) and raw attn out of PSUM, then normalize:
        # rb = D broadcast to 64 partitions via K=1 PE matmul, fast recip
        at = at_of[sup]
        cp = nc.scalar.copy if tail else nc.vector.tensor_copy
        d16 = dn_p.tile([1, HPC, SUP], f16, tag="d16", name=f"d16_{sup}")
        for h in range(HPC):
            cp(out=d16[:, h, :], in_=at[h][HD : HD + 1, :])
        xstg = dn_p.tile([HD, HPC, SUP], f32, tag="xstg", name=f"xstg{sup}")
        for h in range(HPC):
            cp(out=xstg[:, h, :], in_=at[h][0:HD, :])
        rb_ps = [
            aux_ps_p.tile([HD, SUP], f32, tag="aux", name=f"rb{sup}_{h}")
            for h in range(HPC)
        ]
        for h in range(HPC):
            nc.tensor.matmul(
                rb_ps[h], ones_sb, d16[:, h, :], start=True, stop=True
            )
        rb = dn_p.tile([HD, HPC, SUP], f32, tag="rb", name=f"rb{sup}")
        for h in range(HPC):
            nc.vector.reciprocal_approx_fast(out=rb[:, h, :], in_=rb_ps[h])
        for h in range(HPC):
            nc.vector.tensor_mul(
                out=xTn[h][:, ts(sup, SUP)], in0=xstg[:, h, :], in1=rb[:, h, :]
            )

    def out_unit(sup, j):
        # one 128-row output chunk: o = sum_h xTn[h].T @ Wo_h (PSUM-fused)
        st_i = JS * sup + j
        o_ps = [
            aux_ps_p.tile([128, SUP], f32, tag="aux", name=f"o{st_i}_{eh}")
            for eh in range(EMBED // SUP)
        ]
        for eh in range(EMBED // SUP):
            for h in range(HPC):
                nc.tensor.matmul(
                    o_ps[eh],
                    xTn[h][:, ts(st_i, 128)],
                    wo_sb[:, h, ts(eh, SUP)],
                    start=(h == 0), stop=(h == HPC - 1),
                )
        stage = ostg_p.tile([128, EMBED], f16, tag="ostg", name=f"ostg{st_i}")
        for eh in range(EMBED // SUP):
            nc.vector.tensor_copy(out=stage[:, ts(eh, SUP)], in_=o_ps[eh])
        nc.gpsimd.dma_start(out=outp[ts(st_i, 128), :], in_=stage)

    # ---- global conveyor ---------------------------------------------
    # score/Exp stream at (sup, c); the AV stream lags by LAG=8 chunks.
    # At a sup boundary the previous sup's trailing 8 AVs run 2-per-chunk
    # over c=0..3, the drain at c==4, and the new sup's AVs start at c==8
    # -- so the PSUM handoff of the at accumulators never stalls the PE.
    # block-0 projections chase the pair-0 slabs together, e-chunk by
    # e-chunk (q rides the idle at-pool bank; aux ring holds k and v)
    k0_ps = aux_ps_p.tile([128, SUP], f32, tag="aux", name="k0_ps")
    v0_ps = aux_ps_p.tile([128, SUP], f32, tag="aux", name="v0_ps")
    q0_ps = at_ps_p.tile([128, SUP], f32, tag="at0", name="q0_ps")
    for e in range(EC):
        for w_sb, ps in ((wk_sb, k0_ps), (wv_sb, v0_ps), (wq_sb, q0_ps)):
            nc.tensor.matmul(
                ps, w_sb[:, e, :], hTa[:, e, ts(0, SUP)],
                start=(e == 0), stop=(e == EC - 1),
            )
    nc.vector.tensor_scalar(
        out=kT[:, ts(0, SUP)], in0=k0_ps,
        scalar1=bk_sb, scalar2=None, op0=AOT.add,
    )
    vT0_sb = vT_p.tile([128, SUP], f16, tag="vT", name="vT0")
    nc.vector.tensor_scalar(
        out=vT0_sb, in0=v0_ps, scalar1=bv_sb, scalar2=None, op0=AOT.add
    )
    nc.vector.tensor_scalar(
        out=qT[:, ts(0, SUP)], in0=q0_ps,
        scalar1=bq_sb, scalar2=None, op0=AOT.add,
    )
    tp0_ps = aux_ps_p.tile([128, JS, 128], f16, tag="aux", name="tp0_ps")
    for j in range(JS):
        nc.tensor.transpose(tp0_ps[:, j, :], vT0_sb[:, ts(j, 128)], id_sb)
    for j in range(JS):
        dst = vP[:, j, :].rearrange("p (h e) -> p h e", h=HPC)[:, :, 0:HD]
        nc.vector.tensor_copy(
            out=dst, in_=tp0_ps[:, j, :].rearrange("p (h d) -> p h d", h=HPC)
        )

    UNITS = (10, 15, 20, 25)
    for g in range(NSUP * TC):
        sup, c = divmod(g, TC)
        sc_exp(sup, c)
        if sup == 0 and c < 28:
            # phase A, fine-grained: block b = c//4+1 spreads its k/v
            # matmuls 2-at-a-time over the 4 chunks of block b-1, so a
            # late hTa slab never blocks more than one conveyor chunk.
            b = c // JS + 1
            i = c % JS
            if i == 0:
                kv_ps[b] = (
                    aux_ps_p.tile([128, SUP], f32, tag="aux", name=f"k_ps{b}"),
                    aux_ps_p.tile([128, SUP], f32, tag="aux", name=f"v_ps{b}"),
                )
            k_ps, v_ps = kv_ps[b]
            for e in (2 * i, 2 * i + 1):
                nc.tensor.matmul(
                    k_ps, wk_sb[:, e, :], hTa[:, e, ts(b, SUP)],
                    start=(e == 0), stop=(e == EC - 1),
                )
            for e in (2 * i, 2 * i + 1):
                nc.tensor.matmul(
                    v_ps, wv_sb[:, e, :], hTa[:, e, ts(b, SUP)],
                    start=(e == 0), stop=(e == EC - 1),
                )
            if i == JS - 1:
                nc.vector.tensor_scalar(
                    out=kT[:, ts(b, SUP)], in0=k_ps,
                    scalar1=bk_sb, scalar2=None, op0=AOT.add,
                )
                vT_sb = vT_p.tile([128, SUP], f16, tag="vT", name=f"vT{b}")
                nc.vector.tensor_scalar(
                    out=vT_sb, in0=v_ps, scalar1=bv_sb, scalar2=None, op0=AOT.add
                )
                tp_ps = aux_ps_p.tile(
                    [128, JS, 128], f16, tag="aux", name=f"tp_ps{b}"
                )
                for j in range(JS):
                    nc.tensor.transpose(tp_ps[:, j, :], vT_sb[:, ts(j, 128)], id_sb)
                for j in range(JS):
                    t_idx = JS * b + j
                    dst = vP[:, t_idx, :].rearrange(
                        "p (h e) -> p h e", h=HPC
                    )[:, :, 0:HD]
                    nc.vector.tensor_copy(
                        out=dst,
                        in_=tp_ps[:, j, :].rearrange("p (h d) -> p h d", h=HPC),
                    )
                del kv_ps[b]
        if sup >= 1 and c in UNITS:
            out_unit(sup - 1, UNITS.index(c))
        if c == 27 and sup + 1 < NSUP:
            q_proj(sup + 1, half=0)
        if c == 29 and sup + 1 < NSUP:
            q_proj(sup + 1, half=1)
        if sup >= 1:
            if c < JS:
                at_mms(sup - 1, TC - 2 * JS + 2 * c)
                at_mms(sup - 1, TC - 2 * JS + 2 * c + 1)
            elif c == JS:
                drain(sup - 1)
        if c == LAG:
            alloc_at(sup)
        if c >= LAG:
            at_mms(sup, c - LAG)
    # tail: trailing AVs with the final drain interleaved per head so the
    # normalize chain starts as soon as each head's accumulator closes
    S7 = NSUP - 1
    for k in range(JS - 1):
        at_mms(S7, TC - 2 * JS + 2 * k)
        at_mms(S7, TC - 2 * JS + 2 * k + 1)
    at_mms(S7, TC - 2)
    at7 = at_of[S7]
    d16t = dn_p.tile([1, HPC, SUP], f16, tag="d16", name="d16_t")
    xstgt = dn_p.tile([HD, HPC, SUP], f32, tag="xstg", name="xstg_t")
    rbt = dn_p.tile([HD, HPC, SUP], f32, tag="rb", name="rb_t")
    pT31 = pT_of.pop((S7, TC - 1))
    rb_pst = []
    for h in range(HPC):
        nc.tensor.matmul(
            at7[h], vP[:, TC - 1, ts(h, HD + 1)], pT31[:, ts(h, SUP)],
            start=False, stop=True,
        )
        nc.scalar.copy(out=d16t[:, h, :], in_=at7[h][HD : HD + 1, :])
        nc.scalar.copy(out=xstgt[:, h, :], in_=at7[h][0:HD, :])
        ps = aux_ps_p.tile([HD, SUP], f32, tag="aux", name=f"rbt{h}")
        nc.tensor.matmul(ps, ones_sb, d16t[:, h, :], start=True, stop=True)
        rb_pst.append(ps)
    for h in range(HPC):
        nc.vector.reciprocal_approx_fast(out=rbt[:, h, :], in_=rb_pst[h])
        nc.vector.tensor_mul(
            out=xTn[h][:, ts(S7, SUP)], in0=xstgt[:, h, :], in1=rbt[:, h, :]
        )
    for j in range(JS):
        st_i = JS * S7 + j
        sct = sc_ps_p.tile([128, HPC * SUP], f32, tag="sc", name=f"osc{j}")
        stage = ostg_p.tile([128, EMBED], f16, tag="ostg", name=f"ostgt{j}")
        for eh in range(EMBED // SUP):
            for h in range(HPC):
                nc.tensor.matmul(
                    sct[:, ts(eh, SUP)],
                    xTn[h][:, ts(st_i, 128)],
                    wo_sb[:, h, ts(eh, SUP)],
                    start=(h == 0), stop=(h == HPC - 1),
                )
            # drain each 512-wide bank as soon as its head-sum closes;
            # scalar + DVE split, halves DMA'd densely on the idle sync queue
            cp = nc.scalar.copy if eh == 0 else nc.vector.tensor_copy
            cp(out=stage[:, ts(eh, SUP)], in_=sct[:, ts(eh, SUP)])
            nc.sync.dma_start(
                out=outp[ts(st_i, 128), ts(eh, SUP)], in_=stage[:, ts(eh, SUP)]
            )

    st_.close()


def _shards(inputs):
    """Host-side prep: per-core input dicts (head-parallel, Wo row-shard)."""
    hs = np.asarray(inputs["hidden_state"], np.float32)
    Wq = np.asarray(inputs["Wq"], np.float32) * 0.125  # fold 1/sqrt(64); exact
    bq = np.asarray(inputs["bq"], np.float32) * 0.125
    Wk = np.asarray(inputs["Wk"], np.float32)
    bk = np.asarray(inputs["bk"], np.float32)
    Wv = np.asarray(inputs["Wv"], np.float32)
    bv = np.asarray(inputs["bv"], np.float32)
    Wo = np.asarray(inputs["Wo"], np.float32)
    ident = np.eye(128, dtype=np.float16)
    hs16 = np.ascontiguousarray(hs.astype(np.float16))

    in_maps = []
    for c in range(NCORES):
        h0 = HPC * c

        # [H,E,Dh] head-pair -> [E, 2*Dh] -> [128(e), EC, 128] partition-major
        def _w(W):
            w = np.transpose(W[h0 : h0 + HPC], (1, 0, 2)).reshape(EMBED, 128)
            w = w.reshape(EC, 128, 128).transpose(1, 0, 2)  # [e, chunk, d]
            return np.ascontiguousarray(w.astype(np.float16))

        w_qkv = np.stack([_w(Wq), _w(Wk), _w(Wv)], axis=1)
        b_qkv = np.stack(
            [
                bq[h0 : h0 + HPC].reshape(128),
                bk[h0 : h0 + HPC].reshape(128),
                bv[h0 : h0 + HPC].reshape(128),
            ],
            axis=1,
        )
        # Wo rows for this core's heads: [128, E] -> [64, 2, E] head-major
        w_o = np.ascontiguousarray(
            Wo[128 * c : 128 * (c + 1)]
            .reshape(HPC, 64, EMBED)
            .transpose(1, 0, 2)
            .astype(np.float16)
        )
        in_maps.append(
            {
                "hidden_f16": hs16,
                "w_qkv": np.ascontiguousarray(w_qkv),
                "b_qkv": np.ascontiguousarray(b_qkv),
                "w_o": w_o,
                "ident": ident,
            }
        )
    return in_maps


def kernel(**inputs):
    global LAST
    from concourse import bass_utils

    trace = bool(int(os.environ.get("K_TRACE", "0")))
    if trace:
        _install_ntff_shim()

    if "nc" not in _CACHE:
        _CACHE["nc"] = _build()
    nc = _CACHE["nc"]

    in_maps = _shards(inputs)
    res = bass_utils.run_bass_kernel_spmd(
        nc, in_maps, core_ids=list(range(NCORES)), trace=trace
    )
    LAST = res

    out = np.zeros((SEQ, EMBED), np.float64)
    for c in range(NCORES):
        out += res.results[c]["out_p"].astype(np.float64)
    out += np.asarray(inputs["bo"], np.float32).astype(np.float64)
    return out.astype(np.float32)


def _install_ntff_shim():
    """antenv.axon_hooks is absent from this image; recreate it so
    run_bass_kernel_spmd(trace=True) can reach the NTFF profiling hook."""
    import types

    if "antenv.axon_hooks" in sys.modules:
        return
    try:
        if "/root/.axon_site" not in sys.path:
            sys.path.insert(0, "/root/.axon_site")
        from trn_agent_boot.trn_boot import _ntff_profile_via_ctypes

        hook = _ntff_profile_via_ctypes("/opt/axon/libaxon_pjrt.so")
    except Exception:
        hook = None
    mod = types.ModuleType("antenv.axon_hooks")
    mod._hook = hook
    mod.get_axon_ntff_profile_hook = lambda: mod._hook
    mod.set_axon_ntff_profile_hook = lambda h: setattr(mod, "_hook", h)
    sys.modules["antenv.axon_hooks"] = mod
